# revision 8
# baseline (speedup 1.0000x reference)
"""Chunked attention Trainium2 Bass kernel (v2).

Problem: B=2, S=8192, HIDDEN=1024, HEADS=16, HEAD_DIM=64, CHUNK=2048,
OVERLAP=128. Sharding: head-parallel x batch-parallel -> 32 (b,h) jobs,
4 per core on 8 cores. Each core computes full-seq chunked attention for
its 4 heads; the host slices/pre-transposes inputs (bf16) and reassembles
the output.

v2 changes vs v1 (fp32r baseline):
  - bf16 operands end-to-end (fp32 PSUM accumulation), halving DMA.
  - The softmax exp drain (PSUM->SBUF) is the hard bottleneck: ACT is the
    only exp engine (1 elem/lane/cycle @1.2GHz => ~490us/core for the
    75.8M scores a core owns). We split the drain between ACT (true exp,
    bf16 out) and DVE (Schraudolph bit-trick exp: one tensor_scalar
    affine fp32->int16 whose bit pattern IS the bf16 probability),
    load-balanced greedily => ~0.55/0.45 split.
  - PV uses the probs tile as the 128-wide *stationary* operand
    (out[q,65] = pT^T @ [V|1]) instead of V as a 65-wide stationary:
    65 moving rows per 128q x 128k tile instead of 128 -> ~2x less PE
    time for PV; the appended ones column yields the softmax denominator
    in the same accumulation.
  - Device returns UNNORMALIZED [q, 64+1] per q-block; normalization and
    the 128-wide overlap blending happen on the host in fp32.
"""

import sys

if '/opt/trn_rl_repo' not in sys.path:
    sys.path.insert(0, '/opt/trn_rl_repo')

import numpy as np
import ml_dtypes

import concourse.bass as bass
import concourse.mybir as mybir
import concourse.tile as tile
from concourse.bass_utils import run_bass_kernel_spmd

F32 = mybir.dt.float32
BF16 = mybir.dt.bfloat16
I16 = mybir.dt.int16
EXP = mybir.ActivationFunctionType.Exp
COPY = mybir.ActivationFunctionType.Copy
MULT = mybir.AluOpType.mult
ADD = mybir.AluOpType.add

B, S, HIDDEN, HEADS, HD = 2, 8192, 1024, 16, 64
SCALE = 1.0 / 8.0  # 1/sqrt(64)
N_CORES = 8
JOBS = 4  # (b, h) pairs per core
# (q0, Lq, k0, Lk) per chunk; step=1920, overlap=128
CHUNKS = [
    (0, 2048, 0, 2176),
    (1920, 2048, 1792, 2304),
    (3840, 2048, 3712, 2304),
    (5760, 2048, 5632, 2304),
    (7680, 512, 7552, 640),
]
NQB = [lq // 512 for _, lq, _, _ in CHUNKS]  # q-blocks per chunk
QB0 = np.cumsum([0] + NQB).tolist()  # 0,4,8,12,16 ; total 17
NQB_TOT = QB0[-1]
GROUP = 3  # k-tiles per S^T PSUM group (3 banks x2 bufs + opsum x2 = 8)

# Schraudolph bit-trick exp targeting bf16: for x = score*SCALE,
# i16 = rnd(128/ln2 * x + (127*128 - C)); bitcast(i16) ~= exp(x) with
# ~+-3.4% worst-case relative error (C tuned for round AND floor conv).
TRICK_C = 5.375
TRICK_A = float(np.float32(SCALE * 128.0 / np.log(2.0)))
TRICK_B = float(np.float32(127.0 * 128.0 - TRICK_C))

# debug: force all drains to one engine ('act' | 'dve' | None)
FORCE_DRAIN = None


# drain/copy cost model (ns) for greedy ACT/DVE load balancing
def _act_cost(nw):
    return (nw + 172) / 1.2 + 57.0


def _dve_cost(nw):
    return (nw + 120) / 0.96 + 70.0


def _legalize_waits(nc, max_waits=1):
    """walrus in this config rejects >1 sync-wait per instruction: hoist
    excess waits onto injected same-engine NoOps placed just before."""
    cnt = 0
    for f in nc.m.functions:
        for blk in f.blocks:
            il = blk.instructions
            if not any(
                i.sync_info is not None and i.sync_info.on_wait
                and len(i.sync_info.on_wait) > max_waits for i in il
            ):
                continue
            new = []
            for inst in il:
                si = inst.sync_info
                if si is not None and si.on_wait and len(si.on_wait) > max_waits:
                    waits = list(si.on_wait)
                    spill, keep = waits[:-max_waits], waits[-max_waits:]
                    for w in spill:
                        nop = mybir.InstNoOp(
                            name=f"I-wsplit-{cnt}", ins=[], outs=[])
                        cnt += 1
                        nop.engine = inst.engine
                        nop.sync_info = mybir.SyncInfo(on_wait=[w], on_update=[])
                        new.append(nop)
                    inst.sync_info = mybir.SyncInfo(
                        on_wait=keep, on_update=list(si.on_update or []))
                new.append(inst)
            blk.instructions = new
    return cnt


def _build_nc(reps=1):
    nc = bass.Bass()
    qt_in = nc.declare_dram_parameter("qt", [JOBS, 128, S], BF16, isOutput=False)
    kt_in = nc.declare_dram_parameter("kt", [JOBS, 128, S], BF16, isOutput=False)
    # v with ones pre-appended: [j, ktile, p, 65] ; col 64 == 1.0
    v_in = nc.declare_dram_parameter("v", [JOBS, S // 128, 128, 65], BF16,
                                     isOutput=False)
    out = nc.declare_dram_parameter("out", [JOBS, NQB_TOT, 128, 260], F32,
                                    isOutput=True)

    # build the global group list: one entry per (job, chunk, qblock, group)
    work = []  # dicts
    for j in [jj for _ in range(reps) for jj in range(JOBS)]:
        for ci, (q0, lq, k0, lk) in enumerate(CHUNKS):
            nk = lk // 128
            ngroups = (nk + GROUP - 1) // GROUP
            for qb in range(lq // 512):
                for g in range(ngroups):
                    kts = list(range(g * GROUP, min((g + 1) * GROUP, nk)))
                    work.append(dict(
                        j=j, ci=ci, qb=qb, g=g, kts=kts, nk=nk,
                        first=(qb == 0 and g == 0),  # chunk DMA trigger
                        qb_first=(g == 0), qb_last=(g == ngroups - 1)))

    with tile.TileContext(nc) as tc:
        with (
            tc.tile_pool(name="const", bufs=1) as cpool,
            tc.tile_pool(name="ops", bufs=2) as ops,          # qT/kT/vW
            tc.tile_pool(name="probs", bufs=7) as probs,      # pT
            tc.tile_pool(name="opath", bufs=3) as opath,      # o_sb staging
            tc.tile_pool(name="spsum", bufs=2, space="PSUM") as spsum,
            tc.tile_pool(name="onepsum", bufs=2, space="PSUM") as onepsum,
        ):
            zeros = cpool.tile([128, 128], BF16, tag="zeros")
            nc.vector.memset(zeros, 0.0)
            t_act = 0.0  # modeled engine clocks for greedy assignment
            t_dve = 0.0
            cur = {}  # per-chunk live tiles
            opsums = {}  # qblock key -> (opsum tile, o_sb plan)
            pend_pv = None  # (work item, pT tile) lagged by one group

            def emit_qk(w):
                nonlocal cur
                j, ci = w['j'], w['ci']
                q0, lq, k0, lk = CHUNKS[ci]
                if w['first']:
                    nk = w['nk']
                    qT = ops.tile([128, lq], BF16, tag="qT")
                    nc.sync.dma_start(out=qT, in_=qt_in[j, :, q0:q0 + lq])
                    kT = ops.tile([128, lk], BF16, tag="kT")
                    nc.sync.dma_start(out=kT, in_=kt_in[j, :, k0:k0 + lk])
                    vW = ops.tile([128, nk * 65], BF16, tag="vW")
                    nc.sync.dma_start(
                        out=vW.rearrange("p (t e) -> p t e", e=65),
                        in_=v_in[j, k0 // 128:k0 // 128 + nk].rearrange(
                            "t p e -> p t e"))
                    cur = dict(qT=qT, kT=kT, vW=vW)
                qT, kT = cur['qT'], cur['kT']
                qs = slice(w['qb'] * 512, w['qb'] * 512 + 512)
                sp = spsum.tile([128, 512 * GROUP], F32, tag="sp")
                for i, kt in enumerate(w['kts']):
                    rows = slice(64 * (kt % 2), 64 * (kt % 2) + 64)
                    nc.tensor.matmul(
                        sp[:, i * 512:(i + 1) * 512],
                        kT[rows, kt * 128:(kt + 1) * 128],
                        qT[rows, qs],
                        start=True, stop=True,
                        tile_position=(64 * (kt % 2), 0),
                        skip_group_check=True,
                    )
                return sp

            def emit_drain(w, sp):
                nonlocal t_act, t_dve
                nw = 512 * len(w['kts'])
                pT = probs.tile([128, 512 * GROUP], BF16, tag="pT")
                use_act = t_act + _act_cost(nw) <= t_dve + _dve_cost(nw)
                if FORCE_DRAIN is not None:
                    use_act = FORCE_DRAIN == 'act'
                if use_act:
                    t_act += _act_cost(nw)
                    nc.scalar.activation(
                        pT[:, 0:nw], sp[:, 0:nw], EXP, scale=SCALE)
                else:
                    t_dve += _dve_cost(nw)
                    nc.vector.tensor_scalar(
                        pT[:, 0:nw].bitcast(I16), sp[:, 0:nw],
                        TRICK_A, TRICK_B, MULT, ADD)
                return pT

            def emit_pv(w, pT):
                nonlocal t_act, t_dve
                key = (w['j'], w['ci'], w['qb'])
                vW = cur_of[key]['vW']
                if w['qb_first']:
                    opsums[key] = onepsum.tile([128, 260], F32, tag="opsum",
                                               name="opsum")
                    # PSUM start=True zeroing is not region-scoped: one
                    # bank-wide zero matmul, then pure accumulation.
                    nc.tensor.matmul(
                        opsums[key][:, 0:260], zeros, vW[:, 0:260],
                        start=True, stop=False, skip_group_check=True)
                opsum = opsums[key]
                nk = w['nk']
                for i, kt in enumerate(w['kts']):
                    for s in range(4):
                        nc.tensor.matmul(
                            opsum[:, s * 65:(s + 1) * 65],
                            pT[:, i * 512 + s * 128:i * 512 + s * 128 + 128],
                            vW[:, kt * 65:(kt + 1) * 65],
                            start=False, stop=(kt == nk - 1),
                            skip_group_check=True,
                        )
                if w['qb_last']:
                    o_sb = opath.tile([128, 260], F32, tag="osb")
                    c = _act_cost(260)
                    d = _dve_cost(260)
                    if t_act + c <= t_dve + d:
                        t_act += c
                        nc.scalar.activation(o_sb, opsum, COPY)
                    else:
                        t_dve += d
                        nc.vector.tensor_copy(o_sb, opsum)
                    qbg = QB0[w['ci']] + w['qb']
                    nc.sync.dma_start(out=out[w['j'], qbg], in_=o_sb)
                    del opsums[key]

            # pipeline: emit QK(G_i) then PV(G_{i-1}) so drain engines never
            # wait on a q-block/chunk boundary bubble.
            cur_of = {}
            prev = None
            for w in work:
                sp = emit_qk(w)
                cur_of[(w['j'], w['ci'], w['qb'])] = cur
                pT = emit_drain(w, sp)
                if prev is not None:
                    emit_pv(*prev)
                prev = (w, pT)
            emit_pv(*prev)

    _legalize_waits(nc)
    return nc


_NC = None


def _get_nc():
    global _NC
    if _NC is None:
        _NC = _build_nc()
    return _NC


def make_in_maps(query, key_, value):
    """Host-side prep: per-core slices in bf16; Q^T/K^T in [d, seq] layout
    duplicated across both partition halves; V gets a ones column."""
    qh = query.reshape(B, S, HEADS, HD)
    kh = key_.reshape(B, S, HEADS, HD)
    vh = value.reshape(B, S, HEADS, HD)
    qT = np.ascontiguousarray(qh.transpose(0, 2, 3, 1))  # [B, H, D, S]
    kT = np.ascontiguousarray(kh.transpose(0, 2, 3, 1))
    in_maps = []
    for c in range(N_CORES):
        jobs = [(g // HEADS, g % HEADS) for g in range(4 * c, 4 * c + 4)]
        qt_c = np.empty((JOBS, 128, S), ml_dtypes.bfloat16)
        kt_c = np.empty((JOBS, 128, S), ml_dtypes.bfloat16)
        v_c = np.empty((JOBS, S // 128, 128, 65), ml_dtypes.bfloat16)
        for jj, (b, h) in enumerate(jobs):
            qt_c[jj, 0:64] = qT[b, h]
            qt_c[jj, 64:128] = qT[b, h]
            kt_c[jj, 0:64] = kT[b, h]
            kt_c[jj, 64:128] = kT[b, h]
            v_c[jj, :, :, 0:64] = vh[b, :, h].reshape(S // 128, 128, HD)
            v_c[jj, :, :, 64] = 1.0
        in_maps.append({"qt": qt_c, "kt": kt_c, "v": v_c})
    return in_maps


def assemble_out(results):
    """Host: per-chunk softmax division + overlap-band blending (fp32,
    mirrors the reference's merge), then scatter into [B, S, HIDDEN]."""
    wt = np.linspace(1.0, 0.0, 128).astype(np.float32)  # prev-chunk tail
    wh = np.linspace(0.0, 1.0, 128).astype(np.float32)  # cur-chunk head
    denom = (wt + wh) + np.float32(1e-10)
    a = (wt / denom).astype(np.float32)[:, None]
    bb = (wh / denom).astype(np.float32)[:, None]

    out = np.empty((B, S, HIDDEN), dtype=np.float32)
    for c in range(N_CORES):
        oc = results[c]["out"]  # [4, 17, 128, 260]
        for jj, g in enumerate(range(4 * c, 4 * c + 4)):
            b, h = g // HEADS, g % HEADS
            # [17,128,4,65] -> [17,4,128,65] -> [8704, 65] in chunk-concat
            # q order (qb major, then subtile, then partition)
            x = oc[jj].reshape(NQB_TOT, 128, 4, 65).transpose(0, 2, 1, 3)
            x = np.ascontiguousarray(x).reshape(NQB_TOT * 512, 65)
            on_all = x[:, 0:64] / x[:, 64:65]
            full = np.empty((S, HD), np.float32)
            prev_tail = None
            for ci, (q0, lq, k0, lk) in enumerate(CHUNKS):
                on = on_all[QB0[ci] * 512:QB0[ci] * 512 + lq]
                lo = 0
                if ci > 0:
                    full[q0:q0 + 128] = prev_tail * a + on[0:128] * bb
                    lo = 128
                hi = lq
                if ci < len(CHUNKS) - 1:
                    hi = lq - 128
                    prev_tail = on[lq - 128:lq]
                full[q0 + lo:q0 + hi] = on[lo:hi]
            out[b, :, h * HD:(h + 1) * HD] = full
    return out


def kernel(query, key, value):
    query = np.asarray(query, dtype=np.float32)
    key_ = np.asarray(key, dtype=np.float32)
    value = np.asarray(value, dtype=np.float32)
    nc = _get_nc()
    in_maps = make_in_maps(query, key_, value)
    res = run_bass_kernel_spmd(nc, in_maps, list(range(N_CORES)))
    return assemble_out(res.results)


# revision 14
# speedup vs baseline: 1.0241x; 1.0241x over previous
"""Chunked attention Trainium2 Bass kernel (v2).

Problem: B=2, S=8192, HIDDEN=1024, HEADS=16, HEAD_DIM=64, CHUNK=2048,
OVERLAP=128. Sharding: head-parallel x batch-parallel -> 32 (b,h) jobs,
4 per core on 8 cores. Each core computes full-seq chunked attention for
its 4 heads; the host slices/pre-transposes inputs (bf16) and reassembles
the output.

v2 changes vs v1 (fp32r baseline):
  - bf16 operands end-to-end (fp32 PSUM accumulation), halving DMA.
  - The softmax exp drain (PSUM->SBUF) is the hard bottleneck: ACT is the
    only exp engine (1 elem/lane/cycle @1.2GHz => ~490us/core for the
    75.8M scores a core owns). We split the drain between ACT (true exp,
    bf16 out) and DVE (Schraudolph bit-trick exp: one tensor_scalar
    affine fp32->int16 whose bit pattern IS the bf16 probability),
    load-balanced greedily => ~0.55/0.45 split.
  - PV uses the probs tile as the 128-wide *stationary* operand
    (out[q,65] = pT^T @ [V|1]) instead of V as a 65-wide stationary:
    65 moving rows per 128q x 128k tile instead of 128 -> ~2x less PE
    time for PV; the appended ones column yields the softmax denominator
    in the same accumulation.
  - Device returns UNNORMALIZED [q, 64+1] per q-block; normalization and
    the 128-wide overlap blending happen on the host in fp32.
"""

import sys

if '/opt/trn_rl_repo' not in sys.path:
    sys.path.insert(0, '/opt/trn_rl_repo')

import numpy as np
import ml_dtypes

import concourse.bass as bass
import concourse.mybir as mybir
import concourse.tile as tile
from concourse.bass_utils import run_bass_kernel_spmd

F32 = mybir.dt.float32
BF16 = mybir.dt.bfloat16
I16 = mybir.dt.int16
EXP = mybir.ActivationFunctionType.Exp
COPY = mybir.ActivationFunctionType.Copy
MULT = mybir.AluOpType.mult
ADD = mybir.AluOpType.add

B, S, HIDDEN, HEADS, HD = 2, 8192, 1024, 16, 64
SCALE = 1.0 / 8.0  # 1/sqrt(64)
N_CORES = 8
JOBS = 4  # (b, h) pairs per core
# (q0, Lq, k0, Lk) per chunk; step=1920, overlap=128
CHUNKS = [
    (0, 2048, 0, 2176),
    (1920, 2048, 1792, 2304),
    (3840, 2048, 3712, 2304),
    (5760, 2048, 5632, 2304),
    (7680, 512, 7552, 640),
]
NQB = [lq // 512 for _, lq, _, _ in CHUNKS]  # q-blocks per chunk
QB0 = np.cumsum([0] + NQB).tolist()  # 0,4,8,12,16 ; total 17
NQB_TOT = QB0[-1]
SQ = 512 * NQB_TOT  # 8704: sum of chunk Lq (out column space)
GROUP = 3  # k-tiles per S^T PSUM group (3 banks x2 bufs + opsum x2 = 8)

# Schraudolph bit-trick exp targeting bf16: for x = score*SCALE,
# i16 = rnd(128/ln2 * x + (127*128 - C)); bitcast(i16) ~= exp(x) with
# ~+-3.4% worst-case relative error (C tuned for round AND floor conv).
TRICK_C = 5.375
TRICK_A = float(np.float32(SCALE * 128.0 / np.log(2.0)))
TRICK_B = float(np.float32(127.0 * 128.0 - TRICK_C))

# debug: force all drains to one engine ('act' | 'dve' | None)
FORCE_DRAIN = None


# drain/copy cost model (ns) for greedy ACT/DVE load balancing
def _act_cost(nw):
    return (nw + 172) / 1.2 + 57.0


def _dve_cost(nw):
    return (nw + 120) / 0.96 + 70.0


def _legalize_waits(nc, max_waits=1):
    """walrus in this config rejects >1 sync-wait per instruction: hoist
    excess waits onto injected same-engine NoOps placed just before."""
    cnt = 0
    for f in nc.m.functions:
        for blk in f.blocks:
            il = blk.instructions
            if not any(
                i.sync_info is not None and i.sync_info.on_wait
                and len(i.sync_info.on_wait) > max_waits for i in il
            ):
                continue
            new = []
            for inst in il:
                si = inst.sync_info
                if si is not None and si.on_wait and len(si.on_wait) > max_waits:
                    waits = list(si.on_wait)
                    spill, keep = waits[:-max_waits], waits[-max_waits:]
                    for w in spill:
                        nop = mybir.InstNoOp(
                            name=f"I-wsplit-{cnt}", ins=[], outs=[])
                        cnt += 1
                        nop.engine = inst.engine
                        nop.sync_info = mybir.SyncInfo(on_wait=[w], on_update=[])
                        new.append(nop)
                    inst.sync_info = mybir.SyncInfo(
                        on_wait=keep, on_update=list(si.on_update or []))
                new.append(inst)
            blk.instructions = new
    return cnt


def _build_nc(reps=1):
    nc = bass.Bass()
    qt_in = nc.declare_dram_parameter("qt", [JOBS, 128, S], BF16, isOutput=False)
    kt_in = nc.declare_dram_parameter("kt", [JOBS, 128, S], BF16, isOutput=False)
    # v with ones pre-appended: [j, ktile, p, 65] ; col 64 == 1.0
    v_in = nc.declare_dram_parameter("v", [JOBS, S // 128, 128, 65], BF16,
                                     isOutput=False)
    out = nc.declare_dram_parameter("out", [JOBS, 65, SQ], F32,
                                    isOutput=True)

    # build the global group list: one entry per (job, chunk, qblock, group)
    work = []  # dicts
    for j in [jj for _ in range(reps) for jj in range(JOBS)]:
        for ci, (q0, lq, k0, lk) in enumerate(CHUNKS):
            nk = lk // 128
            ngroups = (nk + GROUP - 1) // GROUP
            for qb in range(lq // 512):
                for g in range(ngroups):
                    kts = list(range(g * GROUP, min((g + 1) * GROUP, nk)))
                    work.append(dict(
                        j=j, ci=ci, qb=qb, g=g, kts=kts, nk=nk,
                        first=(qb == 0 and g == 0),  # chunk DMA trigger
                        qb_first=(g == 0), qb_last=(g == ngroups - 1)))

    with tile.TileContext(nc) as tc:
        with (
            tc.tile_pool(name="ops", bufs=2) as ops,          # qT/kT/vW
            tc.tile_pool(name="probs", bufs=7) as probs,      # pT
            tc.tile_pool(name="opath", bufs=3) as opath,      # o_sb staging
            tc.tile_pool(name="spsum", bufs=2, space="PSUM") as spsum,
            tc.tile_pool(name="onepsum", bufs=2, space="PSUM") as onepsum,
        ):
            t_act = 0.0  # modeled engine clocks for greedy assignment
            t_dve = 0.0
            cur = {}  # per-chunk live tiles
            opsums = {}  # qblock key -> (opsum tile, o_sb plan)
            pend_pv = None  # (work item, pT tile) lagged by one group

            def emit_qk(w):
                nonlocal cur
                j, ci = w['j'], w['ci']
                q0, lq, k0, lk = CHUNKS[ci]
                if w['first']:
                    nk = w['nk']
                    qT = ops.tile([128, lq], BF16, tag="qT")
                    nc.sync.dma_start(out=qT, in_=qt_in[j, :, q0:q0 + lq])
                    kT = ops.tile([128, lk], BF16, tag="kT")
                    nc.sync.dma_start(out=kT, in_=kt_in[j, :, k0:k0 + lk])
                    vW = ops.tile([128, nk * 65], BF16, tag="vW")
                    nc.sync.dma_start(
                        out=vW.rearrange("p (t e) -> p t e", e=65),
                        in_=v_in[j, k0 // 128:k0 // 128 + nk].rearrange(
                            "t p e -> p t e"))
                    cur = dict(qT=qT, kT=kT, vW=vW)
                qT, kT = cur['qT'], cur['kT']
                qs = slice(w['qb'] * 512, w['qb'] * 512 + 512)
                sp = spsum.tile([128, 512 * GROUP], F32, tag="sp")
                for i, kt in enumerate(w['kts']):
                    rows = slice(64 * (kt % 2), 64 * (kt % 2) + 64)
                    nc.tensor.matmul(
                        sp[:, i * 512:(i + 1) * 512],
                        kT[rows, kt * 128:(kt + 1) * 128],
                        qT[rows, qs],
                        start=True, stop=True,
                        tile_position=(64 * (kt % 2), 0),
                        skip_group_check=True,
                    )
                return sp

            def emit_drain(w, sp):
                nonlocal t_act, t_dve
                nw = 512 * len(w['kts'])
                pT = probs.tile([128, 512 * GROUP], BF16, tag="pT")
                use_act = t_act + _act_cost(nw) <= t_dve + _dve_cost(nw)
                if FORCE_DRAIN is not None:
                    use_act = FORCE_DRAIN == 'act'
                if use_act:
                    t_act += _act_cost(nw)
                    nc.scalar.activation(
                        pT[:, 0:nw], sp[:, 0:nw], EXP, scale=SCALE)
                else:
                    t_dve += _dve_cost(nw)
                    nc.vector.tensor_scalar(
                        pT[:, 0:nw].bitcast(I16), sp[:, 0:nw],
                        TRICK_A, TRICK_B, MULT, ADD)
                return pT

            def emit_pv(w, pT):
                nonlocal t_act, t_dve
                key = (w['j'], w['ci'], w['qb'])
                vW = cur_of[key]['vW']
                if w['qb_first']:
                    opsums[key] = onepsum.tile([128, 512], F32, tag="opsum",
                                               name="opsum")
                opsum = opsums[key]
                nk = w['nk']
                for i, kt in enumerate(w['kts']):
                    nc.tensor.matmul(
                        opsum[0:65, :],
                        vW[:, kt * 65:(kt + 1) * 65],
                        pT[:, i * 512:(i + 1) * 512],
                        start=(kt == 0), stop=(kt == nk - 1),
                        skip_group_check=True,
                    )
                if w['qb_last']:
                    o_sb = opath.tile([65, 512], F32, tag="osb")
                    c = _act_cost(512)
                    d = _dve_cost(512)
                    if t_act + c <= t_dve + d:
                        t_act += c
                        nc.scalar.activation(o_sb, opsum[0:65, :], COPY)
                    else:
                        t_dve += d
                        nc.vector.tensor_copy(o_sb, opsum[0:65, :])
                    c0 = (QB0[w['ci']] + w['qb']) * 512
                    nc.sync.dma_start(out=out[w['j'], :, c0:c0 + 512],
                                      in_=o_sb)
                    del opsums[key]

            # pipeline: emit QK(G_i) then PV(G_{i-1}) so drain engines never
            # wait on a q-block/chunk boundary bubble.
            cur_of = {}
            prev = None
            for w in work:
                sp = emit_qk(w)
                cur_of[(w['j'], w['ci'], w['qb'])] = cur
                pT = emit_drain(w, sp)
                if prev is not None:
                    emit_pv(*prev)
                prev = (w, pT)
            emit_pv(*prev)

    _legalize_waits(nc)
    return nc


_NC = None


def _get_nc():
    global _NC
    if _NC is None:
        _NC = _build_nc()
    return _NC


def make_in_maps(query, key_, value):
    """Host-side prep: per-core slices in bf16; Q^T/K^T in [d, seq] layout
    duplicated across both partition halves; V gets a ones column."""
    qh = query.reshape(B, S, HEADS, HD)
    kh = key_.reshape(B, S, HEADS, HD)
    vh = value.reshape(B, S, HEADS, HD)
    qT = np.ascontiguousarray(qh.transpose(0, 2, 3, 1))  # [B, H, D, S]
    kT = np.ascontiguousarray(kh.transpose(0, 2, 3, 1))
    in_maps = []
    for c in range(N_CORES):
        jobs = [(g // HEADS, g % HEADS) for g in range(4 * c, 4 * c + 4)]
        qt_c = np.empty((JOBS, 128, S), ml_dtypes.bfloat16)
        kt_c = np.empty((JOBS, 128, S), ml_dtypes.bfloat16)
        v_c = np.empty((JOBS, S // 128, 128, 65), ml_dtypes.bfloat16)
        for jj, (b, h) in enumerate(jobs):
            qt_c[jj, 0:64] = qT[b, h]
            qt_c[jj, 64:128] = qT[b, h]
            kt_c[jj, 0:64] = kT[b, h]
            kt_c[jj, 64:128] = kT[b, h]
            v_c[jj, :, :, 0:64] = vh[b, :, h].reshape(S // 128, 128, HD)
            v_c[jj, :, :, 64] = 1.0
        in_maps.append({"qt": qt_c, "kt": kt_c, "v": v_c})
    return in_maps


def assemble_out(results):
    """Host: per-chunk softmax division + overlap-band blending (fp32,
    mirrors the reference's merge), then scatter into [B, S, HIDDEN]."""
    wt = np.linspace(1.0, 0.0, 128).astype(np.float32)  # prev-chunk tail
    wh = np.linspace(0.0, 1.0, 128).astype(np.float32)  # cur-chunk head
    denom = (wt + wh) + np.float32(1e-10)
    a = (wt / denom).astype(np.float32)[:, None]
    bb = (wh / denom).astype(np.float32)[:, None]

    out = np.empty((B, S, HIDDEN), dtype=np.float32)
    for c in range(N_CORES):
        oc = results[c]["out"]  # [4, 65, SQ]
        for jj, g in enumerate(range(4 * c, 4 * c + 4)):
            b, h = g // HEADS, g % HEADS
            on_all = (oc[jj, 0:64] / oc[jj, 64:65]).T  # [SQ, 64]
            full = np.empty((S, HD), np.float32)
            prev_tail = None
            for ci, (q0, lq, k0, lk) in enumerate(CHUNKS):
                on = on_all[QB0[ci] * 512:QB0[ci] * 512 + lq]
                lo = 0
                if ci > 0:
                    full[q0:q0 + 128] = prev_tail * a + on[0:128] * bb
                    lo = 128
                hi = lq
                if ci < len(CHUNKS) - 1:
                    hi = lq - 128
                    prev_tail = on[lq - 128:lq]
                full[q0 + lo:q0 + hi] = on[lo:hi]
            out[b, :, h * HD:(h + 1) * HD] = full
    return out


def kernel(query, key, value):
    query = np.asarray(query, dtype=np.float32)
    key_ = np.asarray(key, dtype=np.float32)
    value = np.asarray(value, dtype=np.float32)
    nc = _get_nc()
    in_maps = make_in_maps(query, key_, value)
    res = run_bass_kernel_spmd(nc, in_maps, list(range(N_CORES)))
    return assemble_out(res.results)
